# revision 24
# baseline (speedup 1.0000x reference)
"""Self-attention (sigmoid attention) Bass kernel for Trainium2, SPMD on 8 cores.

Problem: B=4, S=1024, F=256, H=8
  q = (X @ Wq).reshape(b,s,f,h); k,v likewise (self-attention)
  attn = sigmoid(sqrt(F) * q.kT) per (b,h);  wv = attn @ v
  out = relu(wv_flat @ Wo)

Sharding: data-parallel over (batch, seq-half): core c handles batch c//2,
query rows [half*512, half*512+512). K/V are computed per-core for the full
batch sequence (duplicated across the 2 cores sharing a batch) — no
collectives needed.

Per-core layout (head-contiguous permuted weights, prepared on host):
  xT   [256,1024]  X[b]^T (features on partitions)
  xqT  [256, 512]  query-half columns of xT
  Wq/Wk/Wv [8,256,256] (h, f_in, f_head):  W[:, f*H+h] -> [h, :, f]
  Wo   [8,256,256] (h, f_head, n):         Wo[f*H+h, :] -> [h, f, :]
Pipeline per head h: QT=[wq^T x_q]^T-layout, KT, V via projection matmuls;
ST_j = KT_j^T-block @ QT (psum) -> sigmoid*16 -> AT_j; OT = sum_j V_j^T @ AT_j;
out += OT^T @ Wo_h accumulated in PSUM across heads; relu at the end.
"""

import numpy as np

B, S, F, H = 4, 1024, 256, 8
N_CORES = 8
SCALE = 16.0  # sqrt(F)
SQ = S // 2  # query rows per core

# "float32r" = single-pass fp32 matmul mode (4x faster than fp32 on PE for
# moving dim >= 256); numerics validated against the fp32 reference in test.
MM_DT = "float32r"

_CACHE = {}


def _build_nc():
    import concourse.mybir as mybir
    import concourse.tile as tile
    from concourse import bacc
    from concourse.tile_rust import add_dep_helper

    f32 = mybir.dt.float32
    rdt = mybir.dt.float32r if MM_DT == "float32r" else mybir.dt.float32

    def mm(ap):
        return ap

    Sigmoid = mybir.ActivationFunctionType.Sigmoid
    Relu = mybir.ActivationFunctionType.Relu

    nc = bacc.Bacc()
    xqT = nc.declare_dram_parameter("xqT", [F, SQ], rdt, isOutput=False)
    xoT = nc.declare_dram_parameter("xoT", [F, SQ], rdt, isOutput=False)
    Wq = nc.declare_dram_parameter("Wq", [H, F, F], rdt, isOutput=False)
    Wk = nc.declare_dram_parameter("Wk", [H, F, F], rdt, isOutput=False)
    Wv = nc.declare_dram_parameter("Wv", [H, F, F], rdt, isOutput=False)
    Wo = nc.declare_dram_parameter("Wo", [H, F, F], rdt, isOutput=False)
    out_d = nc.declare_dram_parameter("out", [SQ, F], f32, isOutput=True)

    NJ = S // 128  # 8 key-row tiles per head
    NM = SQ // 128  # 4 query-row tiles

    with tile.TileContext(nc) as tc:
        with (
            tc.tile_pool(name="const", bufs=1) as const,
            tc.tile_pool(name="sb", bufs=2) as sb,
            tc.tile_pool(name="osb", bufs=1) as osb,
            tc.tile_pool(name="psA", bufs=4, space="PSUM") as psA,
            tc.tile_pool(name="psB", bufs=2, space="PSUM") as psB,
        ):
            # persistent activations (features on partitions, 2 chunks of 128).
            # The key/value sequence is processed in per-core order
            # [query-half, other-half] — attention's j index is a pure
            # reduction index (sigmoid, not softmax), so any consistent
            # permutation of key rows gives the same output. This avoids
            # loading the full x twice.
            # xq on the sync queue first (QT phase needs it immediately);
            # xo on gpsimd (SWDGE) so it loads in parallel with the weights.
            xq = []
            for kk in range(2):
                t = const.tile([128, SQ], rdt, name=f"xq{kk}", tag=f"xq{kk}")
                nc.sync.dma_start(out=t[:], in_=xqT[kk * 128 : (kk + 1) * 128, :])
                xq.append(t)
            xo = []
            xo_dmas = []
            for kk in range(2):
                t = const.tile([128, SQ], rdt, name=f"xo{kk}", tag=f"xo{kk}")
                d = nc.gpsimd.dma_start(
                    out=t[:], in_=xoT[kk * 128 : (kk + 1) * 128, :]
                )
                xo.append(t)
                xo_dmas.append(d)
            xhalves = [xq, xo]

            # per-query-tile output accumulators in SBUF (f32), updated by
            # DVE adds from per-head PSUM partials
            out_acc = [
                osb.tile([128, F], f32, name=f"oacc{m}", tag=f"oacc{m}")
                for m in range(NM)
            ]

            prev = None
            for h in range(H):
                # head weights: 2 chunks of 128 input-features each.
                # HBM fair-shares bandwidth across concurrent DGE queues, so
                # explicitly sequence transfers in consumption order: the
                # first matmul's inputs (xq+wq) get the full bandwidth, later
                # groups are gated behind them.
                wq, wk, wv, wo = [], [], [], []
                dmas = {}
                for nm, dram, lst, eng in (
                    ("wq", Wq, wq, nc.sync),
                    ("wk", Wk, wk, nc.sync),
                    ("wv", Wv, wv, nc.gpsimd),
                    ("wo", Wo, wo, nc.gpsimd),
                ):
                    for kk in range(2):
                        t = sb.tile([128, F], rdt, name=f"{nm}{kk}", tag=f"{nm}{kk}", bufs=3)
                        d = eng.dma_start(
                            out=t[:], in_=dram[h, kk * 128 : (kk + 1) * 128, :]
                        )
                        dmas[f"{nm}{kk}"] = d
                        lst.append(t)
                if h == 0:
                    gate = dmas["wq1"].ins
                    for d in (dmas["wk0"], dmas["wk1"], xo_dmas[0], xo_dmas[1]):
                        add_dep_helper(d.ins, gate, reason="hbm priority")
                    for d in (dmas["wv0"], dmas["wv1"]):
                        add_dep_helper(d.ins, dmas["wk1"].ins, reason="hbm priority")
                    for d in (dmas["wo0"], dmas["wo1"]):
                        add_dep_helper(d.ins, dmas["wv1"].ins, reason="hbm priority")
                else:
                    gate = prev_w_dma.ins
                    for d in dmas.values():
                        add_dep_helper(d.ins, gate, reason="hbm priority")
                prev_w_dma = dmas["wo1"]

                # QT_h [256 fh, 512 q] as 2 tiles [128, 512]
                qT = []
                for m in range(2):
                    ps = psA.tile([128, SQ], f32, name="psq", tag="psA")
                    for kk in range(2):
                        nc.tensor.matmul(
                            ps[:],
                            mm(wq[kk][:, m * 128 : (m + 1) * 128]),
                            mm(xq[kk][:]),
                            start=(kk == 0),
                            stop=(kk == 1),
                        )
                    t = sb.tile([128, SQ], rdt, name=f"qT{m}", tag=f"qT{m}")
                    nc.vector.tensor_copy(t[:], ps[:])
                    qT.append(t)

                # deferred output projection of the PREVIOUS head (gives the
                # DVE time to finish the ot casts without stalling the PE)
                if prev is not None:
                    p_ot, p_wo, p_h = prev
                    for m in range(NM):
                        ps = psB.tile([128, F], f32, name="psop", tag=f"psB{m % 2}")
                        for kk in range(2):
                            nc.tensor.matmul(
                                ps[:],
                                mm(p_ot[kk][:, m * 128 : (m + 1) * 128]),
                                mm(p_wo[kk][:]),
                                start=(kk == 0),
                                stop=(kk == 1),
                            )
                        if p_h == 0:
                            nc.vector.tensor_copy(out_acc[m][:], ps[:])
                        else:
                            nc.vector.tensor_add(out_acc[m][:], out_acc[m][:], ps[:])

                # KT_h [256 fh, 1024 j] as 2 tiles [128, 1024]
                # (j order = [query-half rows, other-half rows])
                kT = []
                for m in range(2):
                    t = sb.tile([128, S], rdt, name=f"kT{m}", tag=f"kT{m}")
                    for n in range(2):
                        ps = psA.tile([128, 512], f32, name="psk", tag="psA")
                        for kk in range(2):
                            nc.tensor.matmul(
                                ps[:],
                                mm(wk[kk][:, m * 128 : (m + 1) * 128]),
                                mm(xhalves[n][kk][:]),
                                start=(kk == 0),
                                stop=(kk == 1),
                            )
                        nc.vector.tensor_copy(t[:, n * 512 : (n + 1) * 512], ps[:])
                    kT.append(t)

                # V_h [1024 j, 256 fh] as 8 tiles [128, 256] (natural layout)
                v = []
                for j in range(NJ):
                    if j % 2 == 0:
                        ps = psA.tile([128, F], f32, name="psv", tag="psA")
                    else:
                        ps = psB.tile([128, F], f32, name="psv", tag=f"psB{(j // 2) % 2}")
                    xh = xhalves[j // 4]
                    jj = j % 4
                    for kk in range(2):
                        nc.tensor.matmul(
                            ps[:],
                            mm(xh[kk][:, jj * 128 : (jj + 1) * 128]),
                            mm(wv[kk][:]),
                            start=(kk == 0),
                            stop=(kk == 1),
                        )
                    t = sb.tile([128, F], rdt, name=f"v{j}", tag=f"v{j}")
                    nc.vector.tensor_copy(t[:], ps[:])
                    v.append(t)

                # ST_j = [128 j, 512 q] -> sigmoid(16*ST) -> AT_j, with the
                # OT accumulation (OT_h = sum_j V_j^T-block @ AT_j) software-
                # pipelined two j-tiles behind so the PE never waits on the
                # sigmoid ACT.
                at = []
                pso = [
                    psB.tile([128, SQ], f32, name=f"pso{m}", tag=f"psB{m}", bufs=2)
                    for m in range(2)
                ]

                def o_contrib(j):
                    for m in range(2):
                        nc.tensor.matmul(
                            pso[m][:],
                            mm(v[j][:, m * 128 : (m + 1) * 128]),
                            mm(at[j][:]),
                            start=(j == 0),
                            stop=(j == NJ - 1),
                        )

                for j in range(NJ):
                    if j < 4:
                        ps = psA.tile([128, SQ], f32, name="pss", tag="psA")
                    else:
                        ps = psB.tile([128, SQ], f32, name="pss", tag=f"psB{j % 2}")
                    for kk in range(2):
                        nc.tensor.matmul(
                            ps[:],
                            mm(kT[kk][:, j * 128 : (j + 1) * 128]),
                            mm(qT[kk][:]),
                            start=(kk == 0),
                            stop=(kk == 1),
                        )
                    t = sb.tile([128, SQ], rdt, name=f"at{j}", tag=f"at{j}")
                    nc.scalar.activation(t[:], ps[:], Sigmoid, scale=SCALE)
                    at.append(t)
                    if j >= 2:
                        o_contrib(j - 2)
                o_contrib(NJ - 2)
                o_contrib(NJ - 1)

                ot = []
                for m in range(2):
                    t = sb.tile([128, SQ], rdt, name=f"ot{m}", tag=f"ot{m}")
                    nc.vector.tensor_copy(t[:], pso[m][:])
                    ot.append(t)

                prev = (ot, wo, h)

            # final head's output projection; relu+store per tile as soon as
            # that tile's accumulation closes
            p_ot, p_wo, p_h = prev
            for m in range(NM):
                ps = psB.tile([128, F], f32, name="psop", tag=f"psB{m % 2}")
                for kk in range(2):
                    nc.tensor.matmul(
                        ps[:],
                        mm(p_ot[kk][:, m * 128 : (m + 1) * 128]),
                        mm(p_wo[kk][:]),
                        start=(kk == 0),
                        stop=(kk == 1),
                    )
                nc.vector.tensor_add(out_acc[m][:], out_acc[m][:], ps[:])
                t = osb.tile([128, F], f32, name=f"outsb{m}", tag=f"outsb{m}")
                nc.scalar.activation(t[:], out_acc[m][:], Relu)
                nc.sync.dma_start(out=out_d[m * 128 : (m + 1) * 128, :], in_=t[:])

    nc.finalize()
    return nc


def _get_nc():
    if "nc" not in _CACHE:
        _CACHE["nc"] = _build_nc()
    return _CACHE["nc"]


def _prep_weights(Wq, Wk, Wv, Wo):
    # [F, F*H] with column f*H+h  ->  [H, F, F] head-contiguous
    wq = np.ascontiguousarray(Wq.reshape(F, F, H).transpose(2, 0, 1))
    wk = np.ascontiguousarray(Wk.reshape(F, F, H).transpose(2, 0, 1))
    wv = np.ascontiguousarray(Wv.reshape(F, F, H).transpose(2, 0, 1))
    # [F*H, F] with row f*H+h  ->  [H, F, F]
    wo = np.ascontiguousarray(Wo.reshape(F, H, F).transpose(1, 0, 2))
    return wq, wk, wv, wo


def kernel(q_input, Wq, Wk, Wv, Wo, _trace=False):
    from concourse.bass_utils import run_bass_kernel_spmd

    nc = _get_nc()
    wq, wk, wv, wo = _prep_weights(
        np.asarray(Wq, np.float32),
        np.asarray(Wk, np.float32),
        np.asarray(Wv, np.float32),
        np.asarray(Wo, np.float32),
    )
    q_input = np.asarray(q_input, np.float32)

    in_maps = []
    for c in range(N_CORES):
        b, half = c // 2, c % 2
        xT = q_input[b].T
        xqT = np.ascontiguousarray(xT[:, half * SQ : (half + 1) * SQ])
        xoT = np.ascontiguousarray(xT[:, (1 - half) * SQ : (2 - half) * SQ])
        in_maps.append(
            {"xqT": xqT, "xoT": xoT, "Wq": wq, "Wk": wk, "Wv": wv, "Wo": wo}
        )

    res = run_bass_kernel_spmd(nc, in_maps, list(range(N_CORES)), trace=_trace)

    out = np.empty((B, S, F), np.float32)
    for c in range(N_CORES):
        b, half = c // 2, c % 2
        out[b, half * SQ : (half + 1) * SQ, :] = res.results[c]["out"]
    if _trace:
        return out, res
    return out


# revision 25
# speedup vs baseline: 1.0450x; 1.0450x over previous
"""Self-attention (sigmoid attention) Bass kernel for Trainium2, SPMD on 8 cores.

Problem: B=4, S=1024, F=256, H=8
  q = (X @ Wq).reshape(b,s,f,h); k,v likewise (self-attention)
  attn = sigmoid(sqrt(F) * q.kT) per (b,h);  wv = attn @ v
  out = relu(wv_flat @ Wo)

Sharding: data-parallel over (batch, seq-half): core c handles batch c//2,
query rows [half*512, half*512+512). K/V are computed per-core for the full
batch sequence (duplicated across the 2 cores sharing a batch) — no
collectives needed.

Per-core layout (head-contiguous permuted weights, prepared on host):
  xT   [256,1024]  X[b]^T (features on partitions)
  xqT  [256, 512]  query-half columns of xT
  Wq/Wk/Wv [8,256,256] (h, f_in, f_head):  W[:, f*H+h] -> [h, :, f]
  Wo   [8,256,256] (h, f_head, n):         Wo[f*H+h, :] -> [h, f, :]
Pipeline per head h: QT=[wq^T x_q]^T-layout, KT, V via projection matmuls;
ST_j = KT_j^T-block @ QT (psum) -> sigmoid*16 -> AT_j; OT = sum_j V_j^T @ AT_j;
out += OT^T @ Wo_h accumulated in PSUM across heads; relu at the end.
"""

import numpy as np

B, S, F, H = 4, 1024, 256, 8
N_CORES = 8
SCALE = 16.0  # sqrt(F)
SQ = S // 2  # query rows per core

# "float32r" = single-pass fp32 matmul mode (4x faster than fp32 on PE for
# moving dim >= 256); numerics validated against the fp32 reference in test.
MM_DT = "float32r"

_CACHE = {}


def _build_nc():
    import concourse.mybir as mybir
    import concourse.tile as tile
    from concourse import bacc
    from concourse.tile_rust import add_dep_helper

    f32 = mybir.dt.float32
    rdt = mybir.dt.float32r if MM_DT == "float32r" else mybir.dt.float32

    def mm(ap):
        return ap

    Sigmoid = mybir.ActivationFunctionType.Sigmoid
    Relu = mybir.ActivationFunctionType.Relu

    nc = bacc.Bacc()
    xqT = nc.declare_dram_parameter("xqT", [F, SQ], rdt, isOutput=False)
    xoT = nc.declare_dram_parameter("xoT", [F, SQ], rdt, isOutput=False)
    Wq = nc.declare_dram_parameter("Wq", [H, F, F], rdt, isOutput=False)
    Wk = nc.declare_dram_parameter("Wk", [H, F, F], rdt, isOutput=False)
    Wv = nc.declare_dram_parameter("Wv", [H, F, F], rdt, isOutput=False)
    Wo = nc.declare_dram_parameter("Wo", [H, F, F], rdt, isOutput=False)
    out_d = nc.declare_dram_parameter("out", [SQ, F], f32, isOutput=True)

    NJ = S // 128  # 8 key-row tiles per head
    NM = SQ // 128  # 4 query-row tiles

    with tile.TileContext(nc) as tc:
        with (
            tc.tile_pool(name="const", bufs=1) as const,
            tc.tile_pool(name="sb", bufs=2) as sb,
            tc.tile_pool(name="osb", bufs=1) as osb,
            tc.tile_pool(name="psA", bufs=4, space="PSUM") as psA,
            tc.tile_pool(name="psB", bufs=2, space="PSUM") as psB,
        ):
            # persistent activations (features on partitions, 2 chunks of 128).
            # The key/value sequence is processed in per-core order
            # [query-half, other-half] — attention's j index is a pure
            # reduction index (sigmoid, not softmax), so any consistent
            # permutation of key rows gives the same output. This avoids
            # loading the full x twice.
            # xq on the sync queue first (QT phase needs it immediately);
            # xo on gpsimd (SWDGE) so it loads in parallel with the weights.
            xq = []
            for kk in range(2):
                t = const.tile([128, SQ], rdt, name=f"xq{kk}", tag=f"xq{kk}")
                nc.sync.dma_start(out=t[:], in_=xqT[kk * 128 : (kk + 1) * 128, :])
                xq.append(t)
            xo = []
            xo_dmas = []
            for kk in range(2):
                t = const.tile([128, SQ], rdt, name=f"xo{kk}", tag=f"xo{kk}")
                d = nc.gpsimd.dma_start(
                    out=t[:], in_=xoT[kk * 128 : (kk + 1) * 128, :]
                )
                xo.append(t)
                xo_dmas.append(d)
            xhalves = [xq, xo]

            # per-query-tile output accumulators in SBUF (f32), updated by
            # DVE adds from per-head PSUM partials
            out_acc = [
                osb.tile([128, F], f32, name=f"oacc{m}", tag=f"oacc{m}")
                for m in range(NM)
            ]

            prev = None
            for h in range(H):
                # head weights: 2 chunks of 128 input-features each.
                # HBM fair-shares bandwidth across concurrent DGE queues, so
                # explicitly sequence transfers in consumption order: the
                # first matmul's inputs (xq+wq) get the full bandwidth, later
                # groups are gated behind them.
                wq, wk, wv, wo = [], [], [], []
                dmas = {}
                for nm, dram, lst, eng in (
                    ("wq", Wq, wq, nc.sync),
                    ("wk", Wk, wk, nc.sync),
                    ("wv", Wv, wv, nc.gpsimd),
                    ("wo", Wo, wo, nc.gpsimd),
                ):
                    for kk in range(2):
                        t = sb.tile([128, F], rdt, name=f"{nm}{kk}", tag=f"{nm}{kk}", bufs=3)
                        d = eng.dma_start(
                            out=t[:], in_=dram[h, kk * 128 : (kk + 1) * 128, :]
                        )
                        dmas[f"{nm}{kk}"] = d
                        lst.append(t)
                if h == 0:
                    gate = dmas["wq1"].ins
                    for d in (dmas["wk0"], dmas["wk1"], xo_dmas[0], xo_dmas[1]):
                        add_dep_helper(d.ins, gate, reason="hbm priority")
                    for d in (dmas["wv0"], dmas["wv1"]):
                        add_dep_helper(d.ins, dmas["wk1"].ins, reason="hbm priority")
                    for d in (dmas["wo0"], dmas["wo1"]):
                        add_dep_helper(d.ins, dmas["wv1"].ins, reason="hbm priority")
                else:
                    gate = prev_w_dma.ins
                    for d in dmas.values():
                        add_dep_helper(d.ins, gate, reason="hbm priority")
                prev_w_dma = dmas["wo1"]

                # QT_h [256 fh, 512 q] as 2 tiles [128, 512]
                qT = []
                for m in range(2):
                    ps = psA.tile([128, SQ], f32, name="psq", tag="psA")
                    for kk in range(2):
                        nc.tensor.matmul(
                            ps[:],
                            mm(wq[kk][:, m * 128 : (m + 1) * 128]),
                            mm(xq[kk][:]),
                            start=(kk == 0),
                            stop=(kk == 1),
                        )
                    t = sb.tile([128, SQ], rdt, name=f"qT{m}", tag=f"qT{m}")
                    nc.vector.tensor_copy(t[:], ps[:])
                    qT.append(t)

                # deferred output projection of the PREVIOUS head (gives the
                # DVE time to finish the ot casts without stalling the PE)
                if prev is not None:
                    p_ot, p_wo, p_h = prev
                    for m in range(NM):
                        ps = psB.tile([128, F], f32, name="psop", tag=f"psB{m % 2}")
                        for kk in range(2):
                            nc.tensor.matmul(
                                ps[:],
                                mm(p_ot[kk][:, m * 128 : (m + 1) * 128]),
                                mm(p_wo[kk][:]),
                                start=(kk == 0),
                                stop=(kk == 1),
                            )
                        if p_h == 0:
                            nc.vector.tensor_copy(out_acc[m][:], ps[:])
                        else:
                            nc.vector.tensor_add(out_acc[m][:], out_acc[m][:], ps[:])

                # KT_h [256 fh, 1024 j] as 2 tiles [128, 1024]
                # (j order = [query-half rows, other-half rows])
                kT = []
                for m in range(2):
                    t = sb.tile([128, S], rdt, name=f"kT{m}", tag=f"kT{m}")
                    for n in range(2):
                        ps = psA.tile([128, 512], f32, name="psk", tag="psA")
                        for kk in range(2):
                            nc.tensor.matmul(
                                ps[:],
                                mm(wk[kk][:, m * 128 : (m + 1) * 128]),
                                mm(xhalves[n][kk][:]),
                                start=(kk == 0),
                                stop=(kk == 1),
                            )
                        nc.vector.tensor_copy(
                            t[:, n * 512 : n * 512 + 256], ps[:, 0:256]
                        )
                        nc.scalar.copy(
                            t[:, n * 512 + 256 : (n + 1) * 512], ps[:, 256:512]
                        )
                    kT.append(t)

                # V_h [1024 j, 256 fh] as 8 tiles [128, 256] (natural layout)
                v = []
                for j in range(NJ):
                    if j % 2 == 0:
                        ps = psA.tile([128, F], f32, name="psv", tag="psA")
                    else:
                        ps = psB.tile([128, F], f32, name="psv", tag=f"psB{(j // 2) % 2}")
                    xh = xhalves[j // 4]
                    jj = j % 4
                    for kk in range(2):
                        nc.tensor.matmul(
                            ps[:],
                            mm(xh[kk][:, jj * 128 : (jj + 1) * 128]),
                            mm(wv[kk][:]),
                            start=(kk == 0),
                            stop=(kk == 1),
                        )
                    t = sb.tile([128, F], rdt, name=f"v{j}", tag=f"v{j}")
                    nc.vector.tensor_copy(t[:], ps[:])
                    v.append(t)

                # ST_j = [128 j, 512 q] -> sigmoid(16*ST) -> AT_j, with the
                # OT accumulation (OT_h = sum_j V_j^T-block @ AT_j) software-
                # pipelined two j-tiles behind so the PE never waits on the
                # sigmoid ACT.
                at = []
                pso = [
                    psB.tile([128, SQ], f32, name=f"pso{m}", tag=f"psB{m}", bufs=2)
                    for m in range(2)
                ]

                def o_contrib(j):
                    for m in range(2):
                        nc.tensor.matmul(
                            pso[m][:],
                            mm(v[j][:, m * 128 : (m + 1) * 128]),
                            mm(at[j][:]),
                            start=(j == 0),
                            stop=(j == NJ - 1),
                        )

                for j in range(NJ):
                    if j < 4:
                        ps = psA.tile([128, SQ], f32, name="pss", tag="psA")
                    else:
                        ps = psB.tile([128, SQ], f32, name="pss", tag=f"psB{j % 2}")
                    for kk in range(2):
                        nc.tensor.matmul(
                            ps[:],
                            mm(kT[kk][:, j * 128 : (j + 1) * 128]),
                            mm(qT[kk][:]),
                            start=(kk == 0),
                            stop=(kk == 1),
                        )
                    t = sb.tile([128, SQ], rdt, name=f"at{j}", tag=f"at{j}")
                    nc.scalar.activation(t[:], ps[:], Sigmoid, scale=SCALE)
                    at.append(t)
                    if j >= 2:
                        o_contrib(j - 2)
                o_contrib(NJ - 2)
                o_contrib(NJ - 1)

                ot = []
                for m in range(2):
                    t = sb.tile([128, SQ], rdt, name=f"ot{m}", tag=f"ot{m}")
                    nc.vector.tensor_copy(t[:], pso[m][:])
                    ot.append(t)

                prev = (ot, wo, h)

            # final head's output projection; relu+store per tile as soon as
            # that tile's accumulation closes
            p_ot, p_wo, p_h = prev
            for m in range(NM):
                ps = psB.tile([128, F], f32, name="psop", tag=f"psB{m % 2}")
                for kk in range(2):
                    nc.tensor.matmul(
                        ps[:],
                        mm(p_ot[kk][:, m * 128 : (m + 1) * 128]),
                        mm(p_wo[kk][:]),
                        start=(kk == 0),
                        stop=(kk == 1),
                    )
                nc.vector.tensor_add(out_acc[m][:], out_acc[m][:], ps[:])
                t = osb.tile([128, F], f32, name=f"outsb{m}", tag=f"outsb{m}")
                nc.scalar.activation(t[:], out_acc[m][:], Relu)
                nc.sync.dma_start(out=out_d[m * 128 : (m + 1) * 128, :], in_=t[:])

    nc.finalize()
    return nc


def _get_nc():
    if "nc" not in _CACHE:
        _CACHE["nc"] = _build_nc()
    return _CACHE["nc"]


def _prep_weights(Wq, Wk, Wv, Wo):
    # [F, F*H] with column f*H+h  ->  [H, F, F] head-contiguous
    wq = np.ascontiguousarray(Wq.reshape(F, F, H).transpose(2, 0, 1))
    wk = np.ascontiguousarray(Wk.reshape(F, F, H).transpose(2, 0, 1))
    wv = np.ascontiguousarray(Wv.reshape(F, F, H).transpose(2, 0, 1))
    # [F*H, F] with row f*H+h  ->  [H, F, F]
    wo = np.ascontiguousarray(Wo.reshape(F, H, F).transpose(1, 0, 2))
    return wq, wk, wv, wo


def kernel(q_input, Wq, Wk, Wv, Wo, _trace=False):
    from concourse.bass_utils import run_bass_kernel_spmd

    nc = _get_nc()
    wq, wk, wv, wo = _prep_weights(
        np.asarray(Wq, np.float32),
        np.asarray(Wk, np.float32),
        np.asarray(Wv, np.float32),
        np.asarray(Wo, np.float32),
    )
    q_input = np.asarray(q_input, np.float32)

    in_maps = []
    for c in range(N_CORES):
        b, half = c // 2, c % 2
        xT = q_input[b].T
        xqT = np.ascontiguousarray(xT[:, half * SQ : (half + 1) * SQ])
        xoT = np.ascontiguousarray(xT[:, (1 - half) * SQ : (2 - half) * SQ])
        in_maps.append(
            {"xqT": xqT, "xoT": xoT, "Wq": wq, "Wk": wk, "Wv": wv, "Wo": wo}
        )

    res = run_bass_kernel_spmd(nc, in_maps, list(range(N_CORES)), trace=_trace)

    out = np.empty((B, S, F), np.float32)
    for c in range(N_CORES):
        b, half = c // 2, c % 2
        out[b, half * SQ : (half + 1) * SQ, :] = res.results[c]["out"]
    if _trace:
        return out, res
    return out


# revision 26
# speedup vs baseline: 1.0512x; 1.0059x over previous
"""Self-attention (sigmoid attention) Bass kernel for Trainium2, SPMD on 8 cores.

Problem: B=4, S=1024, F=256, H=8
  q = (X @ Wq).reshape(b,s,f,h); k,v likewise (self-attention)
  attn = sigmoid(sqrt(F) * q.kT) per (b,h);  wv = attn @ v
  out = relu(wv_flat @ Wo)

Sharding: data-parallel over (batch, seq-half): core c handles batch c//2,
query rows [half*512, half*512+512). K/V are computed per-core for the full
batch sequence (duplicated across the 2 cores sharing a batch) — no
collectives needed.

Per-core layout (head-contiguous permuted weights, prepared on host):
  xT   [256,1024]  X[b]^T (features on partitions)
  xqT  [256, 512]  query-half columns of xT
  Wq/Wk/Wv [8,256,256] (h, f_in, f_head):  W[:, f*H+h] -> [h, :, f]
  Wo   [8,256,256] (h, f_head, n):         Wo[f*H+h, :] -> [h, f, :]
Pipeline per head h: QT=[wq^T x_q]^T-layout, KT, V via projection matmuls;
ST_j = KT_j^T-block @ QT (psum) -> sigmoid*16 -> AT_j; OT = sum_j V_j^T @ AT_j;
out += OT^T @ Wo_h accumulated in PSUM across heads; relu at the end.
"""

import numpy as np

B, S, F, H = 4, 1024, 256, 8
N_CORES = 8
SCALE = 16.0  # sqrt(F)
SQ = S // 2  # query rows per core

# "float32r" = single-pass fp32 matmul mode (4x faster than fp32 on PE for
# moving dim >= 256); numerics validated against the fp32 reference in test.
MM_DT = "float32r"

_CACHE = {}


def _build_nc():
    import concourse.mybir as mybir
    import concourse.tile as tile
    from concourse import bacc
    from concourse.tile_rust import add_dep_helper

    f32 = mybir.dt.float32
    rdt = mybir.dt.float32r if MM_DT == "float32r" else mybir.dt.float32

    def mm(ap):
        return ap

    Sigmoid = mybir.ActivationFunctionType.Sigmoid
    Relu = mybir.ActivationFunctionType.Relu

    nc = bacc.Bacc()
    xqT = nc.declare_dram_parameter("xqT", [F, SQ], rdt, isOutput=False)
    xoT = nc.declare_dram_parameter("xoT", [F, SQ], rdt, isOutput=False)
    Wq = nc.declare_dram_parameter("Wq", [H, F, F], rdt, isOutput=False)
    Wk = nc.declare_dram_parameter("Wk", [H, F, F], rdt, isOutput=False)
    Wv = nc.declare_dram_parameter("Wv", [H, F, F], rdt, isOutput=False)
    Wo = nc.declare_dram_parameter("Wo", [H, F, F], rdt, isOutput=False)
    out_d = nc.declare_dram_parameter("out", [SQ, F], f32, isOutput=True)

    NJ = S // 128  # 8 key-row tiles per head
    NM = SQ // 128  # 4 query-row tiles

    with tile.TileContext(nc) as tc:
        with (
            tc.tile_pool(name="const", bufs=1) as const,
            tc.tile_pool(name="sb", bufs=2) as sb,
            tc.tile_pool(name="osb", bufs=1) as osb,
            tc.tile_pool(name="psA", bufs=4, space="PSUM") as psA,
            tc.tile_pool(name="psB", bufs=2, space="PSUM") as psB,
        ):
            # persistent activations (features on partitions, 2 chunks of 128).
            # The key/value sequence is processed in per-core order
            # [query-half, other-half] — attention's j index is a pure
            # reduction index (sigmoid, not softmax), so any consistent
            # permutation of key rows gives the same output. This avoids
            # loading the full x twice.
            # xq on the sync queue first (QT phase needs it immediately);
            # xo on gpsimd (SWDGE) so it loads in parallel with the weights.
            xq = []
            for kk in range(2):
                t = const.tile([128, SQ], rdt, name=f"xq{kk}", tag=f"xq{kk}")
                nc.sync.dma_start(out=t[:], in_=xqT[kk * 128 : (kk + 1) * 128, :])
                xq.append(t)
            xo = []
            xo_dmas = []
            for kk in range(2):
                t = const.tile([128, SQ], rdt, name=f"xo{kk}", tag=f"xo{kk}")
                d = nc.gpsimd.dma_start(
                    out=t[:], in_=xoT[kk * 128 : (kk + 1) * 128, :]
                )
                xo.append(t)
                xo_dmas.append(d)
            xhalves = [xq, xo]

            # per-query-tile output accumulators in SBUF (f32), updated by
            # DVE adds from per-head PSUM partials
            out_acc = [
                osb.tile([128, F], f32, name=f"oacc{m}", tag=f"oacc{m}")
                for m in range(NM)
            ]

            prev = None
            for h in range(H):
                # head weights: 2 chunks of 128 input-features each.
                # HBM fair-shares bandwidth across concurrent DGE queues, so
                # explicitly sequence transfers in consumption order: the
                # first matmul's inputs (xq+wq) get the full bandwidth, later
                # groups are gated behind them.
                wq, wk, wv, wo = [], [], [], []
                dmas = {}
                for nm, dram, lst, eng in (
                    ("wq", Wq, wq, nc.sync),
                    ("wk", Wk, wk, nc.sync),
                    ("wv", Wv, wv, nc.gpsimd),
                    ("wo", Wo, wo, nc.gpsimd),
                ):
                    for kk in range(2):
                        t = sb.tile([128, F], rdt, name=f"{nm}{kk}", tag=f"{nm}{kk}", bufs=3)
                        d = eng.dma_start(
                            out=t[:], in_=dram[h, kk * 128 : (kk + 1) * 128, :]
                        )
                        dmas[f"{nm}{kk}"] = d
                        lst.append(t)
                if h == 0:
                    gate = dmas["wq1"].ins
                    for d in (dmas["wk0"], dmas["wk1"], xo_dmas[0], xo_dmas[1]):
                        add_dep_helper(d.ins, gate, reason="hbm priority")
                    for d in (dmas["wv0"], dmas["wv1"]):
                        add_dep_helper(d.ins, dmas["wk1"].ins, reason="hbm priority")
                    for d in (dmas["wo0"], dmas["wo1"]):
                        add_dep_helper(d.ins, dmas["wv1"].ins, reason="hbm priority")
                else:
                    gate = prev_w_dma.ins
                    for d in dmas.values():
                        add_dep_helper(d.ins, gate, reason="hbm priority")
                prev_w_dma = dmas["wo1"]

                # QT_h [256 fh, 512 q] as 2 tiles [128, 512]
                qT = []
                for m in range(2):
                    ps = psA.tile([128, SQ], f32, name="psq", tag="psA")
                    for kk in range(2):
                        nc.tensor.matmul(
                            ps[:],
                            mm(wq[kk][:, m * 128 : (m + 1) * 128]),
                            mm(xq[kk][:]),
                            start=(kk == 0),
                            stop=(kk == 1),
                        )
                    t = sb.tile([128, SQ], rdt, name=f"qT{m}", tag=f"qT{m}")
                    nc.vector.tensor_copy(t[:], ps[:])
                    qT.append(t)

                # deferred output projection of the PREVIOUS head (gives the
                # DVE time to finish the ot casts without stalling the PE)
                if prev is not None:
                    p_ot, p_wo, p_h = prev
                    for m in range(NM):
                        ps = psB.tile([128, F], f32, name="psop", tag=f"psB{m % 2}")
                        for kk in range(2):
                            nc.tensor.matmul(
                                ps[:],
                                mm(p_ot[kk][:, m * 128 : (m + 1) * 128]),
                                mm(p_wo[kk][:]),
                                start=(kk == 0),
                                stop=(kk == 1),
                            )
                        if p_h == 0:
                            nc.vector.tensor_copy(out_acc[m][:], ps[:])
                        else:
                            nc.vector.tensor_add(out_acc[m][:], out_acc[m][:], ps[:])

                # KT_h [256 fh, 1024 j] as 2 tiles [128, 1024]
                # (j order = [query-half rows, other-half rows])
                kT = []
                for m in range(2):
                    t = sb.tile([128, S], rdt, name=f"kT{m}", tag=f"kT{m}")
                    for n in range(2):
                        ps = psA.tile([128, 512], f32, name="psk", tag="psA")
                        for kk in range(2):
                            nc.tensor.matmul(
                                ps[:],
                                mm(wk[kk][:, m * 128 : (m + 1) * 128]),
                                mm(xhalves[n][kk][:]),
                                start=(kk == 0),
                                stop=(kk == 1),
                            )
                        if n == 0:
                            nc.vector.tensor_copy(
                                t[:, n * 512 : (n + 1) * 512], ps[:]
                            )
                        else:
                            nc.scalar.copy(
                                t[:, n * 512 : (n + 1) * 512], ps[:]
                            )
                    kT.append(t)

                # V_h [1024 j, 256 fh] as 8 tiles [128, 256] (natural layout)
                v = []
                for j in range(NJ):
                    if j % 2 == 0:
                        ps = psA.tile([128, F], f32, name="psv", tag="psA")
                    else:
                        ps = psB.tile([128, F], f32, name="psv", tag=f"psB{(j // 2) % 2}")
                    xh = xhalves[j // 4]
                    jj = j % 4
                    for kk in range(2):
                        nc.tensor.matmul(
                            ps[:],
                            mm(xh[kk][:, jj * 128 : (jj + 1) * 128]),
                            mm(wv[kk][:]),
                            start=(kk == 0),
                            stop=(kk == 1),
                        )
                    t = sb.tile([128, F], rdt, name=f"v{j}", tag=f"v{j}")
                    nc.vector.tensor_copy(t[:], ps[:])
                    v.append(t)

                # ST_j = [128 j, 512 q] -> sigmoid(16*ST) -> AT_j, with the
                # OT accumulation (OT_h = sum_j V_j^T-block @ AT_j) software-
                # pipelined two j-tiles behind so the PE never waits on the
                # sigmoid ACT.
                at = []
                pso = [
                    psB.tile([128, SQ], f32, name=f"pso{m}", tag=f"psB{m}", bufs=2)
                    for m in range(2)
                ]

                def o_contrib(j):
                    for m in range(2):
                        nc.tensor.matmul(
                            pso[m][:],
                            mm(v[j][:, m * 128 : (m + 1) * 128]),
                            mm(at[j][:]),
                            start=(j == 0),
                            stop=(j == NJ - 1),
                        )

                for j in range(NJ):
                    if j < 4:
                        ps = psA.tile([128, SQ], f32, name="pss", tag="psA")
                    else:
                        ps = psB.tile([128, SQ], f32, name="pss", tag=f"psB{j % 2}")
                    for kk in range(2):
                        nc.tensor.matmul(
                            ps[:],
                            mm(kT[kk][:, j * 128 : (j + 1) * 128]),
                            mm(qT[kk][:]),
                            start=(kk == 0),
                            stop=(kk == 1),
                        )
                    t = sb.tile([128, SQ], rdt, name=f"at{j}", tag=f"at{j}")
                    nc.scalar.activation(t[:], ps[:], Sigmoid, scale=SCALE)
                    at.append(t)
                    if j >= 2:
                        o_contrib(j - 2)
                o_contrib(NJ - 2)
                o_contrib(NJ - 1)

                ot = []
                for m in range(2):
                    t = sb.tile([128, SQ], rdt, name=f"ot{m}", tag=f"ot{m}")
                    nc.vector.tensor_copy(t[:], pso[m][:])
                    ot.append(t)

                prev = (ot, wo, h)

            # final head's output projection; relu+store per tile as soon as
            # that tile's accumulation closes
            p_ot, p_wo, p_h = prev
            for m in range(NM):
                ps = psB.tile([128, F], f32, name="psop", tag=f"psB{m % 2}")
                for kk in range(2):
                    nc.tensor.matmul(
                        ps[:],
                        mm(p_ot[kk][:, m * 128 : (m + 1) * 128]),
                        mm(p_wo[kk][:]),
                        start=(kk == 0),
                        stop=(kk == 1),
                    )
                nc.vector.tensor_add(out_acc[m][:], out_acc[m][:], ps[:])
                t = osb.tile([128, F], f32, name=f"outsb{m}", tag=f"outsb{m}")
                nc.scalar.activation(t[:], out_acc[m][:], Relu)
                nc.sync.dma_start(out=out_d[m * 128 : (m + 1) * 128, :], in_=t[:])

    nc.finalize()
    return nc


def _get_nc():
    if "nc" not in _CACHE:
        _CACHE["nc"] = _build_nc()
    return _CACHE["nc"]


def _prep_weights(Wq, Wk, Wv, Wo):
    # [F, F*H] with column f*H+h  ->  [H, F, F] head-contiguous
    wq = np.ascontiguousarray(Wq.reshape(F, F, H).transpose(2, 0, 1))
    wk = np.ascontiguousarray(Wk.reshape(F, F, H).transpose(2, 0, 1))
    wv = np.ascontiguousarray(Wv.reshape(F, F, H).transpose(2, 0, 1))
    # [F*H, F] with row f*H+h  ->  [H, F, F]
    wo = np.ascontiguousarray(Wo.reshape(F, H, F).transpose(1, 0, 2))
    return wq, wk, wv, wo


def kernel(q_input, Wq, Wk, Wv, Wo, _trace=False):
    from concourse.bass_utils import run_bass_kernel_spmd

    nc = _get_nc()
    wq, wk, wv, wo = _prep_weights(
        np.asarray(Wq, np.float32),
        np.asarray(Wk, np.float32),
        np.asarray(Wv, np.float32),
        np.asarray(Wo, np.float32),
    )
    q_input = np.asarray(q_input, np.float32)

    in_maps = []
    for c in range(N_CORES):
        b, half = c // 2, c % 2
        xT = q_input[b].T
        xqT = np.ascontiguousarray(xT[:, half * SQ : (half + 1) * SQ])
        xoT = np.ascontiguousarray(xT[:, (1 - half) * SQ : (2 - half) * SQ])
        in_maps.append(
            {"xqT": xqT, "xoT": xoT, "Wq": wq, "Wk": wk, "Wv": wv, "Wo": wo}
        )

    res = run_bass_kernel_spmd(nc, in_maps, list(range(N_CORES)), trace=_trace)

    out = np.empty((B, S, F), np.float32)
    for c in range(N_CORES):
        b, half = c // 2, c % 2
        out[b, half * SQ : (half + 1) * SQ, :] = res.results[c]["out"]
    if _trace:
        return out, res
    return out
